# revision 1
# baseline (speedup 1.0000x reference)
"""Chamfer distance loss on 8 Trainium2 NeuronCores (Bass/Tile).

Contract: kernel(pred, target) takes the FULL inputs
  pred   [8, 4096, 3] fp32
  target [8, 4096, 3] fp32
and returns the full output: scalar fp32 (mean chamfer distance over batch).

Strategy: data-parallel over B across the 8 cores (one batch element per
core). Per core, distances d(p_n,q_m) = |p|^2+|q|^2-2p.q are produced by a
K=5 augmented matmul that directly emits PAIR sums and differences over
consecutive target (resp. pred) points:
    s[n,k]  = d(n,2k) + d(n,2k+1)
    dl[n,k] = d(n,2k) - d(n,2k+1)
so min(d1,d2) = (s - |dl|)/2. ScalarE computes |dl| (PSUM->SBUF), and a
runtime-registered custom VectorE op computes (|dl|-s)*0.5 with a fused
max-accumulate, yielding negated row minima in one 1x pass — half the
VectorE traffic of reducing raw distances. Both chamfer directions run as
mirrored orientations. Final reduction: column-collect of negated rowmins,
free-axis max + sum, cross-partition sum via a ones matmul.

Matmuls use tile_position row packing (4 concurrent K=5 matmuls in row
groups 0/32/64/96, one PSUM bank each).
"""

import numpy as np

P = 128
N = 4096
KAUG = 5
NPAIR = N // 2
NT = N // P
UNITS = 2
UCOLS = NPAIR // UNITS
HALF = 512
B = 8

_CACHED = {}


# --------------------------------------------------------------------------- #
# custom DVE op: out = (in1 - in0) * imm2 ; accum_out = max_k out            #
# --------------------------------------------------------------------------- #
def _get_pairmin_op():
    if "op" in _CACHED:
        return _CACHED["op"]
    import concourse.dve_ops as dve_ops
    from concourse.dve_spec import AluOp, C2, Spec, Src0, Src1, _has_src1
    from concourse.dve_spec import lower as dve_lower
    from concourse.dve_uop import DveOpSpec

    name = "CHAMFER_NEG_PAIRMIN"
    if name in dve_ops._SUB_OPCODE_FOR_NAME:
        op = next(o for o in dve_ops.OPS if o.name == name)
        _CACHED["op"] = op
        return op

    def ref(in0, in1, s0, s1, imm2):
        b = ((in1.astype(np.float32) - in0.astype(np.float32)) * imm2).astype(
            np.float32
        )
        red = b.reshape(b.shape[0], -1).max(axis=-1, keepdims=True)
        return b, red

    spec = Spec(body=(Src1 - Src0) * C2, accum=AluOp.MAX, reference=ref)
    row = dve_ops._CUSTOM_DVE_ROW_BASE + len(dve_ops.OPS)
    shas = {}
    for ver in ("v3", "v4"):
        tmp = DveOpSpec(
            name=name, opcode=row, uops=dve_lower(spec, ver=ver),
            rd1_en=_has_src1(spec),
        )
        shas[ver] = tmp.sha(ver)
    op = dve_ops.DveOp(name, spec, subdim=False, uops_sha=shas)
    dve_ops.OPS.append(op)
    dve_ops.CUSTOM_DVE_SPECS[name] = spec
    dve_ops._SUB_OPCODE_FOR_NAME[name] = row
    _CACHED["op"] = op
    return op


# --------------------------------------------------------------------------- #
# Bass program                                                                #
# --------------------------------------------------------------------------- #
def build_nc(loop_n=1):
    import concourse.mybir as mybir
    import concourse.tile as tile
    from concourse import bacc

    pairmin_op = _get_pairmin_op()
    f32 = mybir.dt.float32
    nc = bacc.Bacc(name="chamfer")

    a_dram = [
        nc.dram_tensor("a_pred", [KAUG, N], f32, kind="ExternalInput"),
        nc.dram_tensor("a_tgt", [KAUG, N], f32, kind="ExternalInput"),
    ]
    bs_dram = [
        nc.dram_tensor("bs_a", [KAUG, NPAIR], f32, kind="ExternalInput"),
        nc.dram_tensor("bs_b", [KAUG, NPAIR], f32, kind="ExternalInput"),
    ]
    bd_dram = [
        nc.dram_tensor("bd_a", [KAUG, NPAIR], f32, kind="ExternalInput"),
        nc.dram_tensor("bd_b", [KAUG, NPAIR], f32, kind="ExternalInput"),
    ]
    out_dram = nc.dram_tensor("out", [1, 1], f32, kind="ExternalOutput")

    with tile.TileContext(nc) as tc:
        with (
            tc.tile_pool(name="const", bufs=1) as cpool,
            tc.tile_pool(name="work", bufs=3) as wpool,
            tc.tile_pool(name="psum", bufs=2, space="PSUM") as ppool,
        ):
            a_stack = []
            for d in range(2):
                t = cpool.tile([P, N], f32, tag=f"astack{d}", name=f"astack{d}")
                for g in range(4):
                    nc.sync.dma_start(
                        out=t[32 * g : 32 * g + KAUG, :], in_=a_dram[d][:, :]
                    )
                a_stack.append(t)

            b_stack = {}
            for d in range(2):
                for u in range(UNITS):
                    t = cpool.tile(
                        [P, HALF], f32, tag=f"bstack{d}{u}", name=f"bstack{d}{u}"
                    )
                    for g in range(4):
                        src = bs_dram[d] if g < 2 else bd_dram[d]
                        koff = u * UCOLS + (g % 2) * HALF
                        nc.sync.dma_start(
                            out=t[32 * g : 32 * g + KAUG, :],
                            in_=src[:, koff : koff + HALF],
                        )
                    b_stack[(d, u)] = t

            rowmins = [
                cpool.tile([P, NT * UNITS], f32, tag=f"rm{d}", name=f"rm{d}")
                for d in range(2)
            ]

            def emit_body():
                for d in range(2):
                    for i in range(NT):
                        for u in range(UNITS):
                            s_ps = ppool.tile([P, UCOLS], f32, tag="s", name="s_ps")
                            d_ps = ppool.tile([P, UCOLS], f32, tag="d", name="d_ps")
                            for g in range(4):
                                dst = s_ps if g < 2 else d_ps
                                nc.tensor.matmul(
                                    out=dst[:, (g % 2) * HALF : (g % 2 + 1) * HALF],
                                    lhsT=a_stack[d][
                                        32 * g : 32 * g + KAUG, P * i : P * (i + 1)
                                    ],
                                    rhs=b_stack[(d, u)][32 * g : 32 * g + KAUG, :],
                                    start=True,
                                    stop=True,
                                    tile_position=(32 * g, 0),
                                )
                            absd = wpool.tile([P, UCOLS], f32, tag="absd", name="absd")
                            nc.scalar.activation(
                                out=absd[:, :],
                                in_=d_ps[:, :],
                                func=mybir.ActivationFunctionType.Abs,
                            )
                            scratch = wpool.tile(
                                [P, UCOLS], f32, tag="scratch", name="scratch"
                            )
                            nc.vector._custom_dve(
                                pairmin_op,
                                out=scratch[:, :],
                                in0=s_ps[:, :],
                                in1=absd[:, :],
                                imm2=0.5,
                                accum_out=rowmins[d][
                                    :, i * UNITS + u : i * UNITS + u + 1
                                ],
                            )

            if loop_n == 0:
                emit_body()
            else:
                with tc.For_i(0, loop_n, 1):
                    emit_body()

            ones = cpool.tile([P, 1], f32, tag="ones", name="ones")
            nc.vector.memset(ones[:, :], 1.0)
            partials = cpool.tile([P, 2], f32, tag="partials", name="partials")
            for d in range(2):
                tmin = cpool.tile([P, NT], f32, tag=f"tmin{d}", name=f"tmin{d}")
                nc.vector.tensor_reduce(
                    out=tmin[:, :],
                    in_=rowmins[d].rearrange("p (i u) -> p i u", u=UNITS),
                    axis=mybir.AxisListType.X,
                    op=mybir.AluOpType.max,
                )
                nc.vector.tensor_reduce(
                    out=partials[:, d : d + 1],
                    in_=tmin[:, :],
                    axis=mybir.AxisListType.X,
                    op=mybir.AluOpType.add,
                )
            fin_ps = ppool.tile([1, 2], f32, tag="s", name="fin_ps")
            nc.tensor.matmul(
                out=fin_ps[:, :],
                lhsT=ones[:, :],
                rhs=partials[:, :],
                start=True,
                stop=True,
            )
            res = cpool.tile([1, 2], f32, tag="res", name="res")
            nc.vector.tensor_scalar_mul(res[:, :], fin_ps[:, :], -1.0 / N)
            res2 = cpool.tile([1, 1], f32, tag="res2", name="res2")
            nc.vector.tensor_reduce(
                out=res2[:, :],
                in_=res[:, :],
                axis=mybir.AxisListType.X,
                op=mybir.AluOpType.add,
            )
            nc.sync.dma_start(out=out_dram[:, :], in_=res2[:, :])

    nc.compile()
    return nc


# --------------------------------------------------------------------------- #
# host-side prep                                                              #
# --------------------------------------------------------------------------- #
def _aug_a(x):
    n2 = (x * x).sum(1)
    return np.ascontiguousarray(
        np.stack([n2, np.ones_like(n2), x[:, 0], x[:, 1], x[:, 2]], 0)
    ).astype(np.float32)


def _pair_b(y):
    n2 = (y * y).sum(1).astype(np.float32)
    e, o = y[0::2], y[1::2]
    n2e, n2o = n2[0::2], n2[1::2]
    m = y.shape[0] // 2
    bs = np.stack(
        [
            np.full(m, 2.0, np.float32),
            n2e + n2o,
            -2.0 * (e[:, 0] + o[:, 0]),
            -2.0 * (e[:, 1] + o[:, 1]),
            -2.0 * (e[:, 2] + o[:, 2]),
        ],
        0,
    )
    bd = np.stack(
        [
            np.zeros(m, np.float32),
            n2e - n2o,
            -2.0 * (e[:, 0] - o[:, 0]),
            -2.0 * (e[:, 1] - o[:, 1]),
            -2.0 * (e[:, 2] - o[:, 2]),
        ],
        0,
    )
    return (
        np.ascontiguousarray(bs).astype(np.float32),
        np.ascontiguousarray(bd).astype(np.float32),
    )


def prep_core_inputs(pred_b, target_b):
    bs_a, bd_a = _pair_b(target_b)
    bs_b, bd_b = _pair_b(pred_b)
    return {
        "a_pred": _aug_a(pred_b),
        "a_tgt": _aug_a(target_b),
        "bs_a": bs_a,
        "bd_a": bd_a,
        "bs_b": bs_b,
        "bd_b": bd_b,
    }


# --------------------------------------------------------------------------- #
# public entry point                                                          #
# --------------------------------------------------------------------------- #
def kernel(pred: np.ndarray, target: np.ndarray) -> np.ndarray:
    from concourse import bass_utils

    pred = np.asarray(pred, dtype=np.float32)
    target = np.asarray(target, dtype=np.float32)
    assert pred.shape == (B, N, 3) and target.shape == (B, N, 3), (
        pred.shape,
        target.shape,
    )

    if "nc" not in _CACHED:
        _CACHED["nc"] = build_nc(loop_n=1)
    nc = _CACHED["nc"]

    in_maps = [prep_core_inputs(pred[b], target[b]) for b in range(B)]
    res = bass_utils.run_bass_kernel_spmd(nc, in_maps, core_ids=list(range(B)))
    vals = [float(r["out"][0, 0]) for r in res.results]
    return np.float32(np.mean(vals))


# revision 2
# speedup vs baseline: 1.4450x; 1.4450x over previous
"""Chamfer distance loss on 8 Trainium2 NeuronCores (Bass/Tile).

Contract: kernel(pred, target) takes the FULL inputs
  pred   [8, 4096, 3] fp32
  target [8, 4096, 3] fp32
and returns the full output: scalar fp32 (mean chamfer distance over batch),
matching

  d[b,n,m] = |p_bn|^2 + |q_bm|^2 - 2 p_bn.q_bm
  out = mean_b( mean_n min_m d + mean_m min_n d )

Strategy
--------
Data-parallel over B across the 8 cores (one batch element per core; the
sharding is done host-side by slicing, results averaged host-side).

Per core, a K=5 augmented matmul produces PAIR sums and differences of
distances over consecutive target (resp. pred) points directly:
    s[n,k]  = d(n,2k) + d(n,2k+1)        (rhs column built from q_2k + q_2k+1)
    dl[n,k] = d(n,2k) - d(n,2k+1)        (rhs column built from q_2k - q_2k+1)
so min(d1,d2) = (s - |dl|)/2. ScalarE computes |dl| (PSUM->SBUF) and a
runtime-registered custom VectorE op computes (|dl| - s)*0.5 with a fused
MAX-accumulate, producing negated row minima in a single 1x pass — half the
VectorE traffic of min-reducing raw distances (VectorE is the bottleneck
engine: fp32 reductions run at 1 elem/lane/cycle). Both chamfer directions
run as mirrored orientations (pred rows x target pairs, then target rows x
pred pairs).

Matmuls: fp32 matmul streams at ~2-4 cycles/row on TRN2, so operands are
split hi/lo into float16 (a = hi + lo exactly captures ~22 mantissa bits)
and each distance-pair column is computed as hh + hl + lh accumulated in
fp32 PSUM — fp32-grade precision at ~3x1 cycle/row, further packed 4-wide
with tile_position row groups (0/32/64/96), one PSUM bank per matmul.

PSUM layout: 8 banks = 4 in-flight s tiles + 4 d tiles [128, 512] so the
PE runs 4 units ahead of the VectorE/ScalarE drain.

Final: negated rowmins collected per (tile,unit) column, free-axis max +
sum on VectorE, cross-partition sum via a ones-vector matmul, scaled by
-1/N and summed over the two directions.
"""

import numpy as np

P = 128
N = 4096
KAUG = 5
NPAIR = N // 2
NT = N // P
UNITS = 4
UCOLS = NPAIR // UNITS  # 512
B = 8

_CACHED = {}


# --------------------------------------------------------------------------- #
# custom DVE op: out = (in1 - in0) * imm2 ; accum_out = max_k out             #
# --------------------------------------------------------------------------- #
def _get_pairmin_op():
    if "op" in _CACHED:
        return _CACHED["op"]
    import concourse.dve_ops as dve_ops
    from concourse.dve_spec import AluOp, C2, Spec, Src0, Src1, _has_src1
    from concourse.dve_spec import lower as dve_lower
    from concourse.dve_uop import DveOpSpec

    name = "CHAMFER_NEG_PAIRMIN"
    if name in dve_ops._SUB_OPCODE_FOR_NAME:
        op = next(o for o in dve_ops.OPS if o.name == name)
        _CACHED["op"] = op
        return op

    def ref(in0, in1, s0, s1, imm2):
        b = ((in1.astype(np.float32) - in0.astype(np.float32)) * imm2).astype(
            np.float32
        )
        red = b.reshape(b.shape[0], -1).max(axis=-1, keepdims=True)
        return b, red

    spec = Spec(body=(Src1 - Src0) * C2, accum=AluOp.MAX, reference=ref)
    row = dve_ops._CUSTOM_DVE_ROW_BASE + len(dve_ops.OPS)
    shas = {}
    for ver in ("v3", "v4"):
        tmp = DveOpSpec(
            name=name, opcode=row, uops=dve_lower(spec, ver=ver),
            rd1_en=_has_src1(spec),
        )
        shas[ver] = tmp.sha(ver)
    op = dve_ops.DveOp(name, spec, subdim=False, uops_sha=shas)
    dve_ops.OPS.append(op)
    dve_ops.CUSTOM_DVE_SPECS[name] = spec
    dve_ops._SUB_OPCODE_FOR_NAME[name] = row
    _CACHED["op"] = op
    return op


# --------------------------------------------------------------------------- #
# Bass program (one core; run SPMD on 8)                                      #
# --------------------------------------------------------------------------- #
def build_nc(loop_n=1):
    """loop_n wraps the main body in a For_i — iterations are idempotent.
    loop_n=1 is the production program; larger values are used by test
    harnesses to measure the device body time by slope."""
    import concourse.mybir as mybir
    import concourse.tile as tile
    from concourse import bacc

    pairmin_op = _get_pairmin_op()
    f32 = mybir.dt.float32
    f16 = mybir.dt.float16
    nc = bacc.Bacc(name="chamfer2")

    def dram_in(nm, shape, dt):
        return nc.dram_tensor(nm, shape, dt, kind="ExternalInput")

    a_dram = [[dram_in(f"a{d}_{p}", [KAUG, N], f16) for p in range(2)]
              for d in range(2)]
    bs_dram = [[dram_in(f"bs{d}_{p}", [KAUG, NPAIR], f16) for p in range(2)]
               for d in range(2)]
    bd_dram = [[dram_in(f"bd{d}_{p}", [KAUG, NPAIR], f16) for p in range(2)]
               for d in range(2)]
    out_dram = nc.dram_tensor("out", [1, 1], f32, kind="ExternalOutput")

    with tile.TileContext(nc) as tc:
        with (
            tc.tile_pool(name="const", bufs=1) as cpool,
            tc.tile_pool(name="work", bufs=4) as wpool,
            tc.tile_pool(name="psum", bufs=4, space="PSUM") as ppool,
        ):
            # stationary stacks [128, N]: weight rows replicated at partition
            # offsets 32g for the 4 tile_position row groups
            a_stack = {}
            for d in range(2):
                for p in range(2):
                    t = cpool.tile([P, N], f16, tag=f"as{d}{p}", name=f"as{d}{p}")
                    for g in range(4):
                        nc.sync.dma_start(
                            out=t[32 * g : 32 * g + KAUG, :], in_=a_dram[d][p][:, :]
                        )
                    a_stack[(d, p)] = t

            # moving stacks [128, 512] per (dir, part, unit-pair):
            # row group g streams: g=0 bs@2up | g=1 bd@2up | g=2 bs@2up+1 | g=3 bd@2up+1
            b_stack = {}
            for d in range(2):
                for p in range(2):
                    for up in range(2):
                        t = cpool.tile(
                            [P, UCOLS], f16, tag=f"bs{d}{p}{up}", name=f"bs{d}{p}{up}"
                        )
                        for g in range(4):
                            src = bs_dram[d][p] if g % 2 == 0 else bd_dram[d][p]
                            koff = (2 * up + g // 2) * UCOLS
                            nc.sync.dma_start(
                                out=t[32 * g : 32 * g + KAUG, :],
                                in_=src[:, koff : koff + UCOLS],
                            )
                        b_stack[(d, p, up)] = t

            rowmins = [
                cpool.tile([P, NT * UNITS], f32, tag=f"rm{d}", name=f"rm{d}")
                for d in range(2)
            ]

            def emit_body():
                for d in range(2):
                    for i in range(NT):
                        for up in range(2):
                            tiles = []
                            for j in range(4):
                                which = "s" if j % 2 == 0 else "d"
                                tiles.append(
                                    ppool.tile(
                                        [P, UCOLS], f32, tag=which, name=f"{which}_ps"
                                    )
                                )
                            for g in range(4):
                                dst = tiles[g]
                                for p, (pa, pb) in enumerate(
                                    [(0, 0), (0, 1), (1, 0)]
                                ):
                                    nc.tensor.matmul(
                                        out=dst[:, :],
                                        lhsT=a_stack[(d, pa)][
                                            32 * g : 32 * g + KAUG,
                                            P * i : P * (i + 1),
                                        ],
                                        rhs=b_stack[(d, pb, up)][
                                            32 * g : 32 * g + KAUG, :
                                        ],
                                        start=(p == 0),
                                        stop=(p == 2),
                                        tile_position=(32 * g, 0),
                                    )
                            for j in range(2):
                                s_ps, d_ps = tiles[2 * j], tiles[2 * j + 1]
                                u = 2 * up + j
                                absd = wpool.tile(
                                    [P, UCOLS], f32, tag="absd", name="absd"
                                )
                                nc.scalar.activation(
                                    out=absd[:, :],
                                    in_=d_ps[:, :],
                                    func=mybir.ActivationFunctionType.Abs,
                                )
                                scratch = wpool.tile(
                                    [P, UCOLS], f32, tag="scratch", name="scratch"
                                )
                                nc.vector._custom_dve(
                                    pairmin_op,
                                    out=scratch[:, :],
                                    in0=s_ps[:, :],
                                    in1=absd[:, :],
                                    imm2=0.5,
                                    accum_out=rowmins[d][
                                        :, i * UNITS + u : i * UNITS + u + 1
                                    ],
                                )

            if loop_n == 0:
                emit_body()
            else:
                with tc.For_i(0, loop_n, 1):
                    emit_body()

            ones = cpool.tile([P, 1], f32, tag="ones", name="ones")
            nc.vector.memset(ones[:, :], 1.0)
            partials = cpool.tile([P, 2], f32, tag="partials", name="partials")
            for d in range(2):
                tmin = cpool.tile([P, NT], f32, tag=f"tmin{d}", name=f"tmin{d}")
                nc.vector.tensor_reduce(
                    out=tmin[:, :],
                    in_=rowmins[d].rearrange("p (i u) -> p i u", u=UNITS),
                    axis=mybir.AxisListType.X,
                    op=mybir.AluOpType.max,
                )
                nc.vector.tensor_reduce(
                    out=partials[:, d : d + 1],
                    in_=tmin[:, :],
                    axis=mybir.AxisListType.X,
                    op=mybir.AluOpType.add,
                )
            fin_ps = ppool.tile([1, 2], f32, tag="s", name="fin_ps")
            nc.tensor.matmul(
                out=fin_ps[:, :], lhsT=ones[:, :], rhs=partials[:, :],
                start=True, stop=True,
            )
            res = cpool.tile([1, 2], f32, tag="res", name="res")
            nc.vector.tensor_scalar_mul(res[:, :], fin_ps[:, :], -1.0 / N)
            res2 = cpool.tile([1, 1], f32, tag="res2", name="res2")
            nc.vector.tensor_reduce(
                out=res2[:, :], in_=res[:, :],
                axis=mybir.AxisListType.X, op=mybir.AluOpType.add,
            )
            nc.sync.dma_start(out=out_dram[:, :], in_=res2[:, :])

    nc.compile()
    return nc


# --------------------------------------------------------------------------- #
# host-side prep                                                              #
# --------------------------------------------------------------------------- #
def _aug_a(x):
    n2 = (x * x).sum(1)
    return np.ascontiguousarray(
        np.stack([n2, np.ones_like(n2), x[:, 0], x[:, 1], x[:, 2]], 0)
    ).astype(np.float32)


def _pair_b(y):
    n2 = (y * y).sum(1).astype(np.float32)
    e, o = y[0::2], y[1::2]
    n2e, n2o = n2[0::2], n2[1::2]
    m = y.shape[0] // 2
    bs = np.stack(
        [np.full(m, 2.0, np.float32), n2e + n2o,
         -2.0 * (e[:, 0] + o[:, 0]), -2.0 * (e[:, 1] + o[:, 1]),
         -2.0 * (e[:, 2] + o[:, 2])], 0)
    bd = np.stack(
        [np.zeros(m, np.float32), n2e - n2o,
         -2.0 * (e[:, 0] - o[:, 0]), -2.0 * (e[:, 1] - o[:, 1]),
         -2.0 * (e[:, 2] - o[:, 2])], 0)
    return bs.astype(np.float32), bd.astype(np.float32)


def _split16(x):
    hi = x.astype(np.float16)
    lo = (x - hi.astype(np.float32)).astype(np.float16)
    return np.ascontiguousarray(hi), np.ascontiguousarray(lo)


def prep_core_inputs(pred_b, target_b):
    bs_a, bd_a = _pair_b(target_b)
    bs_b, bd_b = _pair_b(pred_b)
    full = {
        "a0": _aug_a(pred_b), "a1": _aug_a(target_b),
        "bs0": bs_a, "bs1": bs_b, "bd0": bd_a, "bd1": bd_b,
    }
    out = {}
    for k, v in full.items():
        hi, lo = _split16(v)
        out[f"{k}_0"] = hi
        out[f"{k}_1"] = lo
    return out


# --------------------------------------------------------------------------- #
# public entry point                                                          #
# --------------------------------------------------------------------------- #
def kernel(pred: np.ndarray, target: np.ndarray) -> np.ndarray:
    from concourse import bass_utils

    pred = np.asarray(pred, dtype=np.float32)
    target = np.asarray(target, dtype=np.float32)
    assert pred.shape == (B, N, 3) and target.shape == (B, N, 3), (
        pred.shape,
        target.shape,
    )

    if "nc" not in _CACHED:
        _CACHED["nc"] = build_nc(loop_n=1)
    nc = _CACHED["nc"]

    in_maps = [prep_core_inputs(pred[b], target[b]) for b in range(B)]
    res = bass_utils.run_bass_kernel_spmd(nc, in_maps, core_ids=list(range(B)))
    vals = [float(r["out"][0, 0]) for r in res.results]
    return np.float32(np.mean(vals))


# revision 4
# speedup vs baseline: 1.8282x; 1.2652x over previous
"""Chamfer distance loss on 8 Trainium2 NeuronCores (Bass/Tile).

Contract: kernel(pred, target) takes the FULL inputs
  pred   [8, 4096, 3] fp32
  target [8, 4096, 3] fp32
and returns the full output: scalar fp32 (mean chamfer distance over batch),
matching

  d[b,n,m] = |p_bn|^2 + |q_bm|^2 - 2 p_bn.q_bm
  out = mean_b( mean_n min_m d + mean_m min_n d )

Strategy
--------
Data-parallel over B across the 8 cores (one batch element per core; the
sharding is done host-side by slicing, results averaged host-side).

Per core, a K=5 augmented matmul produces PAIR sums and differences of
distances over consecutive target (resp. pred) points directly:
    s[n,k]  = d(n,2k) + d(n,2k+1)        (rhs column built from q_2k + q_2k+1)
    dl[n,k] = d(n,2k) - d(n,2k+1)        (rhs column built from q_2k - q_2k+1)
so min(d1,d2) = (s - |dl|)/2. ScalarE computes |dl| (PSUM->SBUF) and a
runtime-registered custom VectorE op computes (|dl| - s)*0.5 with a fused
MAX-accumulate, producing negated row minima in a single 1x pass — half the
VectorE traffic of min-reducing raw distances (VectorE is the bottleneck
engine: fp32 reductions run at 1 elem/lane/cycle). Both chamfer directions
run as mirrored orientations (pred rows x target pairs, then target rows x
pred pairs).

Matmuls: fp32 matmul streams at ~2-4 cycles/row on TRN2, so operands are
split hi/lo into float16 (a = hi + lo exactly captures ~22 mantissa bits)
and each distance-pair column is computed as hh + hl + lh accumulated in
fp32 PSUM — fp32-grade precision at ~3x1 cycle/row, further packed 4-wide
with tile_position row groups (0/32/64/96), one PSUM bank per matmul.

PSUM layout: 8 banks = 4 in-flight s tiles + 4 d tiles [128, 512] so the
PE runs 4 units ahead of the VectorE/ScalarE drain.

Final: negated rowmins collected per (tile,unit) column, free-axis max +
sum on VectorE, cross-partition sum via a ones-vector matmul, scaled by
-1/N and summed over the two directions.
"""

import numpy as np

P = 128
N = 4096
KAUG = 5
NPAIR = N // 2
NT = N // P
UNITS = 4
UCOLS = NPAIR // UNITS  # 512
B = 8

_CACHED = {}


# --------------------------------------------------------------------------- #
# custom DVE op: out = (in1 - in0) * imm2 ; accum_out = max_k out             #
# --------------------------------------------------------------------------- #
def _get_pairmin_op():
    if "op" in _CACHED:
        return _CACHED["op"]
    import concourse.dve_ops as dve_ops
    from concourse.dve_spec import AluOp, C2, Spec, Src0, Src1, _has_src1
    from concourse.dve_spec import lower as dve_lower
    from concourse.dve_uop import DveOpSpec

    name = "CHAMFER_NEG_PAIRMIN"
    if name in dve_ops._SUB_OPCODE_FOR_NAME:
        op = next(o for o in dve_ops.OPS if o.name == name)
        _CACHED["op"] = op
        return op

    def ref(in0, in1, s0, s1, imm2):
        b = ((in1.astype(np.float32) - in0.astype(np.float32)) * imm2).astype(
            np.float32
        )
        red = b.reshape(b.shape[0], -1).max(axis=-1, keepdims=True)
        return b, red

    spec = Spec(body=(Src1 - Src0) * C2, accum=AluOp.MAX, reference=ref)
    row = dve_ops._CUSTOM_DVE_ROW_BASE + len(dve_ops.OPS)
    shas = {}
    for ver in ("v3", "v4"):
        tmp = DveOpSpec(
            name=name, opcode=row, uops=dve_lower(spec, ver=ver),
            rd1_en=_has_src1(spec),
        )
        shas[ver] = tmp.sha(ver)
    op = dve_ops.DveOp(name, spec, subdim=False, uops_sha=shas)
    dve_ops.OPS.append(op)
    dve_ops.CUSTOM_DVE_SPECS[name] = spec
    dve_ops._SUB_OPCODE_FOR_NAME[name] = row
    _CACHED["op"] = op
    return op


# --------------------------------------------------------------------------- #
# Bass program (one core; run SPMD on 8)                                      #
# --------------------------------------------------------------------------- #
def build_nc(loop_n=1):
    """loop_n wraps the main body in a For_i — iterations are idempotent.
    loop_n=1 is the production program; larger values are used by test
    harnesses to measure the device body time by slope."""
    import concourse.mybir as mybir
    import concourse.tile as tile
    from concourse import bacc

    pairmin_op = _get_pairmin_op()
    f32 = mybir.dt.float32
    f16 = mybir.dt.float16
    nc = bacc.Bacc(name="chamfer2")

    def dram_in(nm, shape, dt):
        return nc.dram_tensor(nm, shape, dt, kind="ExternalInput")

    a_dram = [[dram_in(f"a{d}_{p}", [KAUG, N], f16) for p in range(2)]
              for d in range(2)]
    bs_dram = [[dram_in(f"bs{d}_{p}", [KAUG, NPAIR], f16) for p in range(2)]
               for d in range(2)]
    bd_dram = [[dram_in(f"bd{d}_{p}", [KAUG, NPAIR], f16) for p in range(2)]
               for d in range(2)]
    out_dram = nc.dram_tensor("out", [1, 1], f32, kind="ExternalOutput")

    with tile.TileContext(nc) as tc:
        with (
            tc.tile_pool(name="const", bufs=1) as cpool,
            tc.tile_pool(name="work", bufs=4) as wpool,
            tc.tile_pool(name="psum", bufs=4, space="PSUM") as ppool,
        ):
            # stationary stacks [128, N]: per row group g (partition offset
            # 32g), rows 32g..+5 hold a_hi and 32g+5..+10 hold a_lo, so the
            # K=10 cross-term matmul reads both in one instruction
            a_stack = {}
            for d in range(2):
                t = cpool.tile([P, N], f16, tag=f"as{d}", name=f"as{d}")
                for g in range(4):
                    for p in range(2):
                        nc.sync.dma_start(
                            out=t[32 * g + KAUG * p : 32 * g + KAUG * (p + 1), :],
                            in_=a_dram[d][p][:, :],
                        )
                a_stack[d] = t

            # moving stacks [128, 512] per (dir, unit-pair):
            # row group g streams: g=0 bs@2up | g=1 bd@2up | g=2 bs@2up+1 | g=3 bd@2up+1
            # b1 = b_hi (K=5, pairs a_hi); b2 = [b_lo; b_hi] (K=10, pairs [a_hi; a_lo])
            b_stack = {}
            for d in range(2):
                for up in range(2):
                    t1 = cpool.tile(
                        [P, UCOLS], f16, tag=f"b1_{d}{up}", name=f"b1_{d}{up}"
                    )
                    for g in range(4):
                        srcd = bs_dram[d][0] if g % 2 == 0 else bd_dram[d][0]
                        koff = (2 * up + g // 2) * UCOLS
                        nc.sync.dma_start(
                            out=t1[32 * g : 32 * g + KAUG, :],
                            in_=srcd[:, koff : koff + UCOLS],
                        )
                    b_stack[(d, 1, up)] = t1
                    t2 = cpool.tile(
                        [P, UCOLS], f16, tag=f"b2_{d}{up}", name=f"b2_{d}{up}"
                    )
                    for g in range(4):
                        srch = bs_dram[d][0] if g % 2 == 0 else bd_dram[d][0]
                        srcl = bs_dram[d][1] if g % 2 == 0 else bd_dram[d][1]
                        koff = (2 * up + g // 2) * UCOLS
                        nc.sync.dma_start(
                            out=t2[32 * g : 32 * g + KAUG, :],
                            in_=srcl[:, koff : koff + UCOLS],
                        )
                        nc.sync.dma_start(
                            out=t2[32 * g + KAUG : 32 * g + 2 * KAUG, :],
                            in_=srch[:, koff : koff + UCOLS],
                        )
                    b_stack[(d, 2, up)] = t2

            rowmins = [
                cpool.tile([P, NT * UNITS], f32, tag=f"rm{d}", name=f"rm{d}")
                for d in range(2)
            ]

            def emit_body():
                for d in range(2):
                    for i in range(NT):
                        for up in range(2):
                            tiles = []
                            for j in range(4):
                                which = "s" if j % 2 == 0 else "d"
                                tiles.append(
                                    ppool.tile(
                                        [P, UCOLS], f32, tag=which, name=f"{which}_ps"
                                    )
                                )
                            for g in range(4):
                                dst = tiles[g]
                                nc.tensor.matmul(
                                    out=dst[:, :],
                                    lhsT=a_stack[d][
                                        32 * g : 32 * g + KAUG, P * i : P * (i + 1)
                                    ],
                                    rhs=b_stack[(d, 1, up)][
                                        32 * g : 32 * g + KAUG, :
                                    ],
                                    start=True,
                                    stop=False,
                                    tile_position=(32 * g, 0),
                                )
                                nc.tensor.matmul(
                                    out=dst[:, :],
                                    lhsT=a_stack[d][
                                        32 * g : 32 * g + 2 * KAUG,
                                        P * i : P * (i + 1),
                                    ],
                                    rhs=b_stack[(d, 2, up)][
                                        32 * g : 32 * g + 2 * KAUG, :
                                    ],
                                    start=False,
                                    stop=True,
                                    tile_position=(32 * g, 0),
                                )
                            for j in range(2):
                                s_ps, d_ps = tiles[2 * j], tiles[2 * j + 1]
                                u = 2 * up + j
                                absd = wpool.tile(
                                    [P, UCOLS], f32, tag="absd", name="absd"
                                )
                                nc.scalar.activation(
                                    out=absd[:, :],
                                    in_=d_ps[:, :],
                                    func=mybir.ActivationFunctionType.Abs,
                                )
                                scratch = wpool.tile(
                                    [P, UCOLS], f32, tag="scratch", name="scratch"
                                )
                                nc.vector._custom_dve(
                                    pairmin_op,
                                    out=scratch[:, :],
                                    in0=s_ps[:, :],
                                    in1=absd[:, :],
                                    imm2=0.5,
                                    accum_out=rowmins[d][
                                        :, i * UNITS + u : i * UNITS + u + 1
                                    ],
                                )

            if loop_n == 0:
                emit_body()
            else:
                with tc.For_i(0, loop_n, 1):
                    emit_body()

            ones = cpool.tile([P, 1], f32, tag="ones", name="ones")
            nc.vector.memset(ones[:, :], 1.0)
            partials = cpool.tile([P, 2], f32, tag="partials", name="partials")
            for d in range(2):
                tmin = cpool.tile([P, NT], f32, tag=f"tmin{d}", name=f"tmin{d}")
                nc.vector.tensor_reduce(
                    out=tmin[:, :],
                    in_=rowmins[d].rearrange("p (i u) -> p i u", u=UNITS),
                    axis=mybir.AxisListType.X,
                    op=mybir.AluOpType.max,
                )
                nc.vector.tensor_reduce(
                    out=partials[:, d : d + 1],
                    in_=tmin[:, :],
                    axis=mybir.AxisListType.X,
                    op=mybir.AluOpType.add,
                )
            fin_ps = ppool.tile([1, 2], f32, tag="s", name="fin_ps")
            nc.tensor.matmul(
                out=fin_ps[:, :], lhsT=ones[:, :], rhs=partials[:, :],
                start=True, stop=True,
            )
            res = cpool.tile([1, 2], f32, tag="res", name="res")
            nc.vector.tensor_scalar_mul(res[:, :], fin_ps[:, :], -1.0 / N)
            res2 = cpool.tile([1, 1], f32, tag="res2", name="res2")
            nc.vector.tensor_reduce(
                out=res2[:, :], in_=res[:, :],
                axis=mybir.AxisListType.X, op=mybir.AluOpType.add,
            )
            nc.sync.dma_start(out=out_dram[:, :], in_=res2[:, :])

    nc.compile()
    return nc


# --------------------------------------------------------------------------- #
# host-side prep                                                              #
# --------------------------------------------------------------------------- #
def _aug_a(x):
    n2 = (x * x).sum(1)
    return np.ascontiguousarray(
        np.stack([n2, np.ones_like(n2), x[:, 0], x[:, 1], x[:, 2]], 0)
    ).astype(np.float32)


def _pair_b(y):
    n2 = (y * y).sum(1).astype(np.float32)
    e, o = y[0::2], y[1::2]
    n2e, n2o = n2[0::2], n2[1::2]
    m = y.shape[0] // 2
    bs = np.stack(
        [np.full(m, 2.0, np.float32), n2e + n2o,
         -2.0 * (e[:, 0] + o[:, 0]), -2.0 * (e[:, 1] + o[:, 1]),
         -2.0 * (e[:, 2] + o[:, 2])], 0)
    bd = np.stack(
        [np.zeros(m, np.float32), n2e - n2o,
         -2.0 * (e[:, 0] - o[:, 0]), -2.0 * (e[:, 1] - o[:, 1]),
         -2.0 * (e[:, 2] - o[:, 2])], 0)
    return bs.astype(np.float32), bd.astype(np.float32)


def _split16(x):
    hi = x.astype(np.float16)
    lo = (x - hi.astype(np.float32)).astype(np.float16)
    return np.ascontiguousarray(hi), np.ascontiguousarray(lo)


def prep_core_inputs(pred_b, target_b):
    bs_a, bd_a = _pair_b(target_b)
    bs_b, bd_b = _pair_b(pred_b)
    full = {
        "a0": _aug_a(pred_b), "a1": _aug_a(target_b),
        "bs0": bs_a, "bs1": bs_b, "bd0": bd_a, "bd1": bd_b,
    }
    out = {}
    for k, v in full.items():
        hi, lo = _split16(v)
        out[f"{k}_0"] = hi
        out[f"{k}_1"] = lo
    return out


# --------------------------------------------------------------------------- #
# public entry point                                                          #
# --------------------------------------------------------------------------- #
def kernel(pred: np.ndarray, target: np.ndarray) -> np.ndarray:
    from concourse import bass_utils

    pred = np.asarray(pred, dtype=np.float32)
    target = np.asarray(target, dtype=np.float32)
    assert pred.shape == (B, N, 3) and target.shape == (B, N, 3), (
        pred.shape,
        target.shape,
    )

    if "nc" not in _CACHED:
        _CACHED["nc"] = build_nc(loop_n=1)
    nc = _CACHED["nc"]

    in_maps = [prep_core_inputs(pred[b], target[b]) for b in range(B)]
    res = bass_utils.run_bass_kernel_spmd(nc, in_maps, core_ids=list(range(B)))
    vals = [float(r["out"][0, 0]) for r in res.results]
    return np.float32(np.mean(vals))
